# revision 12
# baseline (speedup 1.0000x reference)
"""Dense dot-product attention (B=4, H=16, S=2048, D=64) on 8 TRN2 NeuronCores.

Sharding: the 64 (b, h) slices are split 8-per-core (batch+head parallel, no
communication). Per slice, scores are computed transposed (S^T[k, q]) so the
softmax numerator exp(S^T) is already laid out as P^T, which then feeds the
PV matmul as the STATIONARY operand with V' = [V | ones] streaming:

  S^T chunk [128k, 512q] = matmul(lhsT=K^T[64d, 128k], rhs=Q^T[64d, 512q])
  P^T = exp(S^T)                       (ScalarE, PSUM -> SBUF, bf16)
  out[128q, 65] += matmul(lhsT=P^T[128k, 128q], rhs=V'[128k, 65])

so the output accumulates directly in natural [q, d] layout (no output
transpose) with column 64 the softmax denominator; the final divide is a
per-partition reciprocal + tensor_scalar_mul on VectorE straight out of PSUM.
No max-subtraction: scores ~ N(0, 64), |s| < ~55, exp stays in fp32 range and
softmax is shift-invariant.

QK matmuls run in float32r (1 cyc/row at free dim 512); Q^T/K^T are built by
PE transposes of f32r tiles (1.5 cyc/row vs 2.0 for f32). exp writes P^T in
bf16, V' is bf16, so PV streams bf16 at 65 cycles per accumulation step.

The Scalar engine (exp: 262144 elems/partition per core at 1 elem/cycle) is
the bottleneck, so the schedule keeps it streaming: per q-block the 16 k-
chunks are QK'd in 6 groups (3,3,3,3,2,2) alternating two 3-bank PSUM pools
(A,B,A,B,A,B) so the next q-block's first QK group WARs an exp two groups
back, not the one still running. PV groups (and the next slice's Q/K
transposes) are drained as PE filler between QK groups with a 2-group lag.
PSUM: 3 (A) + 3 (B) + 1 (transpose staging) + 1 (out accumulators) = 8 banks.
"""

import sys

sys.path.insert(0, "/opt/trn_rl_repo")

from collections import deque
from contextlib import ExitStack

import numpy as np

import bass_rust
import concourse.bass as bass
import concourse.tile as tile
from concourse import mybir
from concourse.bass_utils import run_bass_kernel_spmd
from concourse.masks import make_identity

B, H, S, D = 4, 16, 2048, 64
NCORES = 8
NS = (B * H) // NCORES  # slices per core
NCH = S // 128          # 16 key chunks per slice
NQB = S // 512          # 4 q-blocks per slice
F32 = mybir.dt.float32
F32R = mybir.dt.float32r
EXP = mybir.ActivationFunctionType.Exp
BF16 = mybir.dt.bfloat16

# QK chunk groups per q-block: (start_chunk, n_chunks), alternating PSUM
# pools A/B (3 banks each).
QK_GROUPS = ((0, 3), (3, 3), (6, 3), (9, 3), (12, 2), (14, 2))

# Group indices whose exp runs as pow(e, x) on the GPSIMD engine instead of
# ScalarE (offloads the softmax-exp bottleneck if Pool pow throughput allows).
POOL_EXP_GROUPS: frozenset = frozenset()


_ENGINE_NS = {
    mybir.EngineType.SP: "sync",
    mybir.EngineType.PE: "tensor",
    mybir.EngineType.Activation: "scalar",
    mybir.EngineType.DVE: "vector",
    mybir.EngineType.Pool: "gpsimd",
}


def _fix_multiwait(nc):
    """This walrus build accepts only one sync wait per instruction. Tile can
    emit several; move extra waits onto preceding single-wait same-engine
    nops (queue stalls on the nop, same semantics)."""
    n_fixed = 0
    for f in nc.m.functions:
        for bb in f.blocks:
            il = bb.instructions
            for ins in list(il):
                si = ins.sync_info
                if si is None or ins.engine not in _ENGINE_NS:
                    continue
                waits = list(si.on_wait)
                if len(waits) <= 1:
                    continue
                ins.sync_info = bass_rust.SyncInfo(
                    on_wait=[waits[-1]], on_update=list(si.on_update)
                )
                eng = getattr(nc, _ENGINE_NS[ins.engine])
                idx = il.index(ins)
                for w in waits[:-1]:
                    nop_ins = eng.nop().ins
                    nop_ins.sync_info = bass_rust.SyncInfo(on_wait=[w], on_update=[])
                    for f2 in nc.m.functions:
                        for bb2 in f2.blocks:
                            il2 = bb2.instructions
                            for kk in range(len(il2) - 1, -1, -1):
                                if il2[kk] is nop_ins:
                                    del il2[kk]
                    il.insert(idx, nop_ins)
                    idx += 1
                n_fixed += 1
    return n_fixed


def _attention_body(ctx: ExitStack, tc: tile.TileContext, q, k, v, o):
    nc = tc.nc

    singles = ctx.enter_context(tc.tile_pool(name="singles", bufs=1))
    nat = ctx.enter_context(tc.tile_pool(name="nat", bufs=2))
    vpool = ctx.enter_context(tc.tile_pool(name="vpool", bufs=2))
    tpool = ctx.enter_context(tc.tile_pool(name="tpool", bufs=2))
    ptp = ctx.enter_context(tc.tile_pool(name="ptp", bufs=2))
    oout = ctx.enter_context(tc.tile_pool(name="oout", bufs=2))
    rp = ctx.enter_context(tc.tile_pool(name="rp", bufs=4))
    psA = ctx.enter_context(tc.tile_pool(name="psA", bufs=1, space="PSUM"))
    psB = ctx.enter_context(tc.tile_pool(name="psB", bufs=1, space="PSUM"))
    pso = ctx.enter_context(tc.tile_pool(name="pso", bufs=1, space="PSUM"))
    psmt = ctx.enter_context(tc.tile_pool(name="psmt", bufs=1, space="PSUM"))

    ident_f = singles.tile([128, 128], F32)
    make_identity(nc, ident_f)
    ident = singles.tile([128, 128], F32R)
    nc.vector.tensor_copy(ident, ident_f)
    etile = None
    if POOL_EXP_GROUPS:
        import math
        etile = singles.tile([128, 2 * 512], F32)
        nc.gpsimd.memset(etile, float(math.e))

    pending_pv = deque()   # PV-group closures, drained one per QK group
    pending_tr = deque()   # next slice's transpose-group closures
    skip = [2]             # initial PV drain lag (groups)

    def emit_epilogue(po, s, qb):
        r = rp.tile([128, 4, 1], F32)
        nc.vector.reciprocal(r, po[:, :, 64:65])
        o_out = oout.tile([128, 4, 64], F32)
        for t in range(4):
            nc.vector.tensor_scalar_mul(o_out[:, t, :], po[:, t, 0:64], r[:, t, :])
        o_re = o[s].rearrange("(n p) d -> p n d", p=128)
        nc.sync.dma_start(out=o_re[:, qb * 4 : (qb + 1) * 4, :], in_=o_out)

    def make_pv(v_sb, pt, s, qb, c0, nch, po_holder):
        def emit():
            if po_holder[0] is None:
                po_holder[0] = pso.tile([128, 4, 65], F32, tag="po", name="po")
            po = po_holder[0]
            # One accumulation group covers the whole po bank (start marks the
            # full 2KB zero-region pending-zero; each tile's first chunk write
            # then overwrites, later chunks accumulate). start/stop only on
            # the very first/last matmul touching the bank this q-block.
            for t in range(4):
                for j in range(nch):
                    c = c0 + j
                    nc.tensor.matmul(
                        out=po[:, t, :],
                        lhsT=pt[:, c * 512 + t * 128 : c * 512 + t * 128 + 128],
                        rhs=v_sb[:, c, :],
                        start=(c == 0 and t == 0),
                        stop=(c == NCH - 1 and t == 3),
                    )
            if c0 + nch == NCH:
                emit_epilogue(po, s, qb)
        return emit

    def load_slice(s, split=False):
        k_nat = nat.tile([128, NCH, 64], F32R, tag="knat")
        k_re = k[s].rearrange("(n p) d -> p n d", p=128)
        q_nat = nat.tile([128, NCH, 64], F32R, tag="qnat")
        q_re = q[s].rearrange("(n p) d -> p n d", p=128)
        if split:
            # First slice: land the chunks the first transpose groups need
            # before the bulk, so the first QK group starts ASAP.
            nc.sync.dma_start(out=k_nat[:, 0:4], in_=k_re[:, 0:4])
            nc.sync.dma_start(out=q_nat[:, 0:4], in_=q_re[:, 0:4])
            nc.sync.dma_start(out=k_nat[:, 4:NCH], in_=k_re[:, 4:NCH])
            nc.sync.dma_start(out=q_nat[:, 4:NCH], in_=q_re[:, 4:NCH])
        else:
            nc.sync.dma_start(out=k_nat, in_=k_re)
            nc.sync.dma_start(out=q_nat, in_=q_re)
        v_f32 = nat.tile([128, NCH, 65], F32, tag="vf32")
        nc.sync.dma_start(
            out=v_f32[:, :, 0:64], in_=v[s].rearrange("(n p) d -> p n d", p=128)
        )
        # ones column + bf16 cast on the idle GPSIMD engine so the DVE
        # queue (transpose copies, epilogues) never head-of-line blocks on
        # the V DMA.
        nc.gpsimd.memset(v_f32[:, :, 64:65], 1.0)
        v_sb = vpool.tile([128, NCH, 65], BF16)
        nc.gpsimd.tensor_copy(v_sb, v_f32)
        qt = tpool.tile([64, S], F32R, tag="qt")
        kt = tpool.tile([64, S], F32R, tag="kt")
        return q_nat, k_nat, v_sb, qt, kt

    def make_tr_groups(q_nat, k_nat, qt, kt):
        groups = {}
        for tn, (nat_t, tt) in (("q", (q_nat, qt)), ("k", (k_nat, kt))):
            for G in range(4):
                def tr(nat_t=nat_t, tt=tt, G=G):
                    stg = psmt.tile([64, 512], F32R, tag="mt")
                    for j in range(4):
                        nc.tensor.transpose(
                            out=stg[:, j * 128 : (j + 1) * 128],
                            in_=nat_t[:, 4 * G + j, :],
                            identity=ident,
                        )
                    nc.vector.tensor_copy(tt[0:64, G * 512 : (G + 1) * 512], stg)
                groups[(tn, G)] = tr
        return groups

    cur = load_slice(0, split=True)
    tr0 = make_tr_groups(cur[0], cur[1], cur[3], cur[4])
    tr0[("k", 0)]()
    tr0[("q", 0)]()
    # Remaining slice-0 transpose groups, emitted just before the first QK
    # group that needs them (k: every qb reads all chunks; q: qb reads its
    # own 512-column window).
    eager_tr = {
        (0, 1): [tr0[("k", 1)]],
        (0, 2): [tr0[("k", 2)]],
        (0, 4): [tr0[("k", 3)]],
        (1, 0): [tr0[("q", 1)]],
        (2, 0): [tr0[("q", 2)]],
        (3, 0): [tr0[("q", 3)]],
    }

    for s in range(NS):
        q_nat, k_nat, v_sb, qt, kt = cur
        if s + 1 < NS:
            cur = load_slice(s + 1)
            trs = make_tr_groups(cur[0], cur[1], cur[3], cur[4])
            pending_tr.extend(trs[(tn, G)] for tn in ("q", "k") for G in range(4))

        for qb in range(NQB):
            pt = ptp.tile([128, NCH * 512], BF16)
            po_holder = [None]
            for gi, (c0, nch) in enumerate(QK_GROUPS):
                if s == 0:
                    for tr in eager_tr.pop((qb, gi), ()):
                        tr()
                pool = psA if gi % 2 == 0 else psB
                ps = pool.tile([128, 3 * 512], F32, tag="ps")
                for j in range(nch):
                    c = c0 + j
                    nc.tensor.matmul(
                        out=ps[:, j * 512 : (j + 1) * 512],
                        lhsT=kt[0:64, c * 128 : (c + 1) * 128],
                        rhs=qt[0:64, qb * 512 : (qb + 1) * 512],
                        start=True,
                        stop=True,
                    )
                if gi in POOL_EXP_GROUPS:
                    nc.gpsimd.tensor_tensor(
                        out=pt[:, c0 * 512 : (c0 + nch) * 512],
                        in0=etile[:, 0 : nch * 512],
                        in1=ps[:, 0 : nch * 512],
                        op=mybir.AluOpType.pow,
                    )
                else:
                    nc.scalar.activation(
                        out=pt[:, c0 * 512 : (c0 + nch) * 512],
                        in_=ps[:, 0 : nch * 512],
                        func=EXP,
                    )
                pending_pv.append(make_pv(v_sb, pt, s, qb, c0, nch, po_holder))
                ndrain = 2 if (s == NS - 1 and qb == NQB - 1) else 1
                for _ in range(ndrain):
                    if skip[0] > 0:
                        skip[0] -= 1
                    elif pending_pv:
                        pending_pv.popleft()()
                if pending_tr and qb >= 2:
                    pending_tr.popleft()()

    while pending_tr:
        pending_tr.popleft()()
    while pending_pv:
        pending_pv.popleft()()


def _build(loop_r=None):
    nc = bass.Bass(num_devices=NCORES)
    q = nc.dram_tensor("q", [NS, S, D], F32R, kind="ExternalInput")
    k = nc.dram_tensor("k", [NS, S, D], F32R, kind="ExternalInput")
    v = nc.dram_tensor("v", [NS, S, D], F32, kind="ExternalInput")
    o = nc.dram_tensor("o", [NS, S, D], F32, kind="ExternalOutput")
    with tile.TileContext(nc) as tc:
        with ExitStack() as ctx:
            if loop_r:
                with tc.For_i(0, loop_r, 1):
                    _attention_body(ctx, tc, q.ap(), k.ap(), v.ap(), o.ap())
            else:
                _attention_body(ctx, tc, q.ap(), k.ap(), v.ap(), o.ap())
    _fix_multiwait(nc)
    return nc


def kernel(Q, K, V, _trace=False, _trace_kwargs=None):
    Qr = np.ascontiguousarray(Q.reshape(NCORES, NS, S, D))
    Kr = np.ascontiguousarray(K.reshape(NCORES, NS, S, D))
    Vr = np.ascontiguousarray(V.reshape(NCORES, NS, S, D))
    nc = _build()
    in_maps = [
        {"q": Qr[i], "k": Kr[i], "v": Vr[i]} for i in range(NCORES)
    ]
    res = run_bass_kernel_spmd(
        nc, in_maps, core_ids=list(range(NCORES)), trace=_trace,
        **(_trace_kwargs or {}),
    )
    out = np.stack([res.results[i]["o"] for i in range(NCORES)], axis=0)
    out = out.reshape(B, H, S, D).astype(np.float32, copy=False)
    if _trace:
        return out, res
    return out


# revision 22
# speedup vs baseline: 1.5129x; 1.5129x over previous
"""Dense dot-product attention (B=4, H=16, S=2048, D=64) on 8 TRN2 NeuronCores.

Sharding: the 64 (b, h) slices are split 8-per-core (batch+head parallel, no
communication). Per slice, scores are computed transposed (S^T[k, q]) so the
softmax numerator exp(S^T) is already laid out as P^T for the P@V matmul:

  S^T chunk [128k, 512q] = matmul(lhsT=K^T[64d, 128k], rhs=Q^T[64d, 512q])
  P^T = exp(S^T)                      (ScalarE, PSUM -> SBUF)
  out'^T [65, 512q] += matmul(lhsT=V'[128k, 65], rhs=P^T[128k, 512q])

where V' = [V | ones] so row 64 of out'^T is the softmax denominator.
No max-subtraction: scores ~ N(0, 64), |s| < ~55, exp stays in fp32 range and
softmax is shift-invariant. Final transpose back to [q, d] on the PE, divide
by the denominator on VectorE, DMA out.

QK matmuls run in float32r (fast fp32 PE path; fp32 proper is 4 cyc/row);
the exp writes P^T in bf16 and V' is bf16, so the P@V side streams bf16.
PV of q-block i is interleaved into the QK-group gaps of block i+1 so the
in-order PE stays busy while QK waits on exp's PSUM WAR (4/2/4/2/4-bank
ping-pong + out' + transpose-staging = 8 PSUM banks).
"""

import sys

sys.path.insert(0, "/opt/trn_rl_repo")

from contextlib import ExitStack

import numpy as np

import bass_rust
import concourse.bass as bass
import concourse.tile as tile
from concourse import mybir
from concourse.bass_utils import run_bass_kernel_spmd
from concourse.masks import make_identity

B, H, S, D = 4, 16, 2048, 64
NCORES = 8
NS = (B * H) // NCORES  # slices per core
NCH = S // 128          # 16 key chunks per slice
NQB = S // 512          # 4 q-blocks per slice
F32 = mybir.dt.float32
F32R = mybir.dt.float32r
EXP = mybir.ActivationFunctionType.Exp
BF16 = mybir.dt.bfloat16

# QK chunk groups per q-block: (start_chunk, n_chunks). Sized so the PSUM
# ping-pong (4-bank + 2-bank) plus out' (1) and transpose staging (1) fit in
# the 8 PSUM banks while ScalarE reads big (2048/1024-elem) spans.
QK_GROUPS = ((0, 4), (4, 2), (6, 4), (10, 2), (12, 4))


_ENGINE_NS = {
    mybir.EngineType.SP: "sync",
    mybir.EngineType.PE: "tensor",
    mybir.EngineType.Activation: "scalar",
    mybir.EngineType.DVE: "vector",
    mybir.EngineType.Pool: "gpsimd",
}


def _fix_multiwait(nc):
    """This walrus build accepts only one sync wait per instruction. Tile can
    emit several; move extra waits onto preceding single-wait same-engine
    nops (queue stalls on the nop, same semantics)."""
    n_fixed = 0
    for f in nc.m.functions:
        for bb in f.blocks:
            il = bb.instructions
            for ins in list(il):
                si = ins.sync_info
                if si is None or ins.engine not in _ENGINE_NS:
                    continue
                waits = list(si.on_wait)
                if len(waits) <= 1:
                    continue
                ins.sync_info = bass_rust.SyncInfo(
                    on_wait=[waits[-1]], on_update=list(si.on_update)
                )
                eng = getattr(nc, _ENGINE_NS[ins.engine])
                idx = il.index(ins)
                for w in waits[:-1]:
                    nop_ins = eng.nop().ins
                    nop_ins.sync_info = bass_rust.SyncInfo(on_wait=[w], on_update=[])
                    for f2 in nc.m.functions:
                        for bb2 in f2.blocks:
                            il2 = bb2.instructions
                            for kk in range(len(il2) - 1, -1, -1):
                                if il2[kk] is nop_ins:
                                    del il2[kk]
                    il.insert(idx, nop_ins)
                    idx += 1
                n_fixed += 1
    return n_fixed


def _attention_body(ctx: ExitStack, tc: tile.TileContext, q, k, v, o, dup=()):
    nc = tc.nc

    singles = ctx.enter_context(tc.tile_pool(name="singles", bufs=1))
    nat = ctx.enter_context(tc.tile_pool(name="nat", bufs=2))
    vpool = ctx.enter_context(tc.tile_pool(name="vpool", bufs=2))
    tpool = ctx.enter_context(tc.tile_pool(name="tpool", bufs=2))
    ptp = ctx.enter_context(tc.tile_pool(name="ptp", bufs=2))
    osb = ctx.enter_context(tc.tile_pool(name="osb", bufs=2))
    oout = ctx.enter_context(tc.tile_pool(name="oout", bufs=2))
    rp = ctx.enter_context(tc.tile_pool(name="rp", bufs=8))
    ps4 = ctx.enter_context(tc.tile_pool(name="ps4", bufs=1, space="PSUM"))
    ps2 = ctx.enter_context(tc.tile_pool(name="ps2", bufs=1, space="PSUM"))
    pso = ctx.enter_context(tc.tile_pool(name="pso", bufs=1, space="PSUM"))
    psmt = ctx.enter_context(tc.tile_pool(name="psmt", bufs=1, space="PSUM"))

    ident = singles.tile([128, 128], F32)
    make_identity(nc, ident)

    # software pipeline: PV + epilogue of q-block i is interleaved between the
    # QK groups of q-block i+1 so the PE has queued work while QK waits on the
    # exp (PSUM WAR) of its own block. state: [v_sb, pt, s, qb, po, next_chunk]
    pending = []

    def emit_pv(nchunks):
        if not pending:
            return
        st = pending[0]
        v_sb, pt, s, qb, po, c0 = st
        if po is None:
            po = pso.tile([65, 512], F32, tag="po")
            st[4] = po
        reps = 2 if "pv" in dup else 1
        hi = min(c0 + nchunks, NCH * reps)
        for ci in range(c0, hi):
            c = ci % NCH
            nc.tensor.matmul(
                out=po[:],
                lhsT=v_sb[:, c, :],
                rhs=pt[:, c * 512 : (c + 1) * 512],
                start=(c == 0),
                stop=(c == NCH - 1),
            )
        st[5] = hi
        if hi < NCH * reps:
            return
        o_sb = osb.tile([65, 512], F32)
        nc.vector.tensor_copy(o_sb, po)
        ot = psmt.tile([128, 4 * 65], F32, tag="mt")
        for i in range(4):
            nc.tensor.transpose(
                out=ot[:, i * 65 : (i + 1) * 65],
                in_=o_sb[:, i * 128 : (i + 1) * 128],
                identity=ident[0:65, 0:65],
            )
        o_out = oout.tile([128, 4, 64], F32)
        for i in range(4):
            r = rp.tile([128, 1], F32)
            nc.vector.reciprocal(r, ot[:, i * 65 + 64 : i * 65 + 65])
            nc.vector.tensor_scalar_mul(
                o_out[:, i, :], ot[:, i * 65 : i * 65 + 64], r
            )
        o_re = o[s].rearrange("(n p) d -> p n d", p=128)
        nc.sync.dma_start(out=o_re[:, qb * 4 : (qb + 1) * 4, :], in_=o_out)
        pending.clear()

    def flush_pending():
        while pending:
            emit_pv(NCH)

    for s in range(NS):
        q_nat = nat.tile([128, NCH, 64], F32, tag="qnat")
        nc.sync.dma_start(out=q_nat, in_=q[s].rearrange("(n p) d -> p n d", p=128))
        k_nat = nat.tile([128, NCH, 64], F32, tag="knat")
        nc.sync.dma_start(out=k_nat, in_=k[s].rearrange("(n p) d -> p n d", p=128))
        v_f32 = nat.tile([128, NCH, 65], F32, tag="vf32")
        nc.sync.dma_start(
            out=v_f32[:, :, 0:64], in_=v[s].rearrange("(n p) d -> p n d", p=128)
        )
        nc.vector.memset(v_f32[:, :, 64:65], 1.0)
        v_sb = vpool.tile([128, NCH, 65], BF16)
        nc.vector.tensor_copy(v_sb, v_f32)

        qt = tpool.tile([64, S], F32R, tag="qt")
        kt = tpool.tile([64, S], F32R, tag="kt")
        for nat_t, tt in ((q_nat, qt), (k_nat, kt)):
            for g in range(4):
                stg = psmt.tile([64, 512], F32, tag="mt")
                for j in range(4):
                    c = 4 * g + j
                    for _rep in range(2 if "tr" in dup else 1):
                        nc.tensor.transpose(
                            out=stg[:, j * 128 : (j + 1) * 128],
                            in_=nat_t[:, c, :],
                            identity=ident,
                        )
                nc.vector.tensor_copy(tt[0:64, g * 512 : (g + 1) * 512], stg)

        for qb in range(NQB):
            pt = ptp.tile([128, NCH * 512], BF16)
            reps = 2 if "pv" in dup else 1
            pv_per_gap = (NCH * reps) // 5
            for c0, nch in QK_GROUPS:
                emit_pv(pv_per_gap)
                ps = (ps4 if nch == 4 else ps2).tile(
                    [128, nch * 512], F32, tag=f"sg{nch}"
                )
                for j in range(nch):
                    c = c0 + j
                    for _rep in range(2 if "qk" in dup else 1):
                        nc.tensor.matmul(
                            out=ps[:, j * 512 : (j + 1) * 512],
                            lhsT=kt[0:64, c * 128 : (c + 1) * 128],
                            rhs=qt[0:64, qb * 512 : (qb + 1) * 512],
                            start=True,
                            stop=True,
                        )
                for _rep in range(2 if "exp" in dup else 1):
                    nc.scalar.activation(
                        out=pt[:, c0 * 512 : (c0 + nch) * 512], in_=ps[:, :], func=EXP
                    )
            flush_pending()
            pending.append([v_sb, pt, s, qb, None, 0])
    flush_pending()


def _build(loop_r=None, dup=()):
    nc = bass.Bass(num_devices=NCORES)
    q = nc.dram_tensor("q", [NS, S, D], F32, kind="ExternalInput")
    k = nc.dram_tensor("k", [NS, S, D], F32, kind="ExternalInput")
    v = nc.dram_tensor("v", [NS, S, D], F32, kind="ExternalInput")
    o = nc.dram_tensor("o", [NS, S, D], F32, kind="ExternalOutput")
    with tile.TileContext(nc) as tc:
        with ExitStack() as ctx:
            if loop_r:
                with tc.For_i(0, loop_r, 1):
                    _attention_body(ctx, tc, q.ap(), k.ap(), v.ap(), o.ap(), dup)
            else:
                _attention_body(ctx, tc, q.ap(), k.ap(), v.ap(), o.ap(), dup)
    _fix_multiwait(nc)
    return nc


def kernel(Q, K, V, _trace=False, _trace_kwargs=None):
    Qr = np.ascontiguousarray(Q.reshape(NCORES, NS, S, D))
    Kr = np.ascontiguousarray(K.reshape(NCORES, NS, S, D))
    Vr = np.ascontiguousarray(V.reshape(NCORES, NS, S, D))
    nc = _build()
    in_maps = [
        {"q": Qr[i], "k": Kr[i], "v": Vr[i]} for i in range(NCORES)
    ]
    res = run_bass_kernel_spmd(
        nc, in_maps, core_ids=list(range(NCORES)), trace=_trace,
        **(_trace_kwargs or {}),
    )
    out = np.stack([res.results[i]["o"] for i in range(NCORES)], axis=0)
    out = out.reshape(B, H, S, D).astype(np.float32, copy=False)
    if _trace:
        return out, res
    return out
